# revision 13
# baseline (speedup 1.0000x reference)
"""Trainium2 Bass kernel for nn_EquivarianceNetwork (grouped 4-layer MLP).

Math (per sample b, TWO_N=16 groups, D=64):
  xr = x.reshape(B, 16, 64)
  scalars[b, n, m] = <xr[b,n], xr[b,m]>          # symmetric 16x16 grid
  per group l: h = tanh(...W0/W1/W2...), coeffs = h @ W3 + b3   # [B, 16]
  out[b, l*64:(l+1)*64] = sum_n coeffs[l,b,n] * xr[b,n]

Distribution: data-parallel over batch across 8 cores (weights replicated).
Per core B_local = 2048.

v6 design (PE floor ~2.1ms at ~1 row/cycle; fp8/DoubleRow measured to give
no real-HW MAC-rate gain, so everything is fp16):
  - All MLP matmuls fp16 (slightly faster rows, half DMA/SBUF, fp32 PSUM).
  - Batch-pair-major loop: weights restreamed per (l, btpair) chunk (2x
    restream ~ 140MB total, ~62GB/s) so the Gram pipeline has ~580us of
    slack per 8 subtiles instead of ~35us -> no PE starvation on scalars.
  - Gram: fp16 2x-mode band mults into one contiguous product buffer
    (big bands on DVE, small bands on GPSIMD), 64->8 fold chain on DVE
    (2x), short 1x strided reduces into the (n,m) scalar grid; mirror
    cols zero (W0 mirror rows zeroed host-side); memset kills NaNs.
  - x resident in SBUF as fp16 (xhall) -> no re-DMA for finals.
  - finals: coeffs broadcast-expanded on ACT (enables DVE 2x multiply),
    fp16 2x tree adds, DVE/GPSIMD split.
  - Deferred tails: per chunk, PE transposes + coeff staging flush at the
    next chunk start; cexp expansion + finals flush mid-chunk so ACT's
    tanh stream is never delayed.
  - PE warmup transposes cover the initial gram pipe-fill (p-state).
"""
import numpy as np
from contextlib import ExitStack
import ml_dtypes

import concourse.bass as bass
import concourse.mybir as mybir
import concourse.tile as tile
from concourse import bacc
from concourse.bass_utils import run_bass_kernel_spmd
from concourse.masks import make_identity

F32 = mybir.dt.float32
F16 = mybir.dt.float16
TANH = mybir.ActivationFunctionType.Tanh

N_CORES = 8
B = 16384
TWO_N = 16
D = 64
B_LOC = B // N_CORES          # 2048
N_SUB = B_LOC // 128          # 16 subtiles of 128 samples
N_BT = B_LOC // 512           # 4 batch tiles of 512 (matmul free dim)
H = 1024                      # hidden width
K_TRI = 136                   # packed upper-triangle band count
CUM = [dl * TWO_N - dl * (dl - 1) // 2 for dl in range(TWO_N + 1)]
SPLIT_DL = 9                  # gram mults: dl < SPLIT_DL on DVE, rest GP


def _build_program():
    nc = bacc.Bacc()

    x = nc.declare_dram_parameter("x", [B_LOC, TWO_N * D], F32, isOutput=False)
    W0g = nc.declare_dram_parameter("W0g", [TWO_N, 256, H], F16,
                                    isOutput=False)
    W12 = nc.declare_dram_parameter("W12", [TWO_N, 2 * H, H], F16,
                                    isOutput=False)
    W3 = nc.declare_dram_parameter("W3", [TWO_N, H, TWO_N], F16,
                                   isOutput=False)
    b0 = nc.declare_dram_parameter("b0", [TWO_N, H], F32, isOutput=False)
    b1 = nc.declare_dram_parameter("b1", [TWO_N, H], F32, isOutput=False)
    b2 = nc.declare_dram_parameter("b2", [TWO_N, H], F32, isOutput=False)
    b3 = nc.declare_dram_parameter("b3", [TWO_N, TWO_N], F32, isOutput=False)
    y = nc.declare_dram_parameter("y", [B_LOC, TWO_N * D], F32, isOutput=True)

    with tile.TileContext(nc) as tc, ExitStack() as ctx:
        pool = lambda *a, **kw: ctx.enter_context(tc.tile_pool(*a, **kw))
        res = pool(name="res", bufs=1)
        xgp = pool(name="xg", bufs=2)
        wk = pool(name="wk", bufs=2)
        sbmp = pool(name="sbm", bufs=16)
        w0p = pool(name="w0", bufs=3)
        w12p = pool(name="w12", bufs=6)
        w3p = pool(name="w3p", bufs=3)
        hp = pool(name="hp", bufs=2)
        pvp = pool(name="pv", bufs=2)
        pgp = pool(name="pg", bufs=2)
        pbp = pool(name="pb", bufs=2)
        cxp = pool(name="cx", bufs=5)
        cfp = pool(name="cf", bufs=32)
        finp = pool(name="fin", bufs=4)
        csbp = pool(name="csb", bufs=3)
        ps = pool(name="ps", bufs=5, space="PSUM")
        tps = pool(name="tp", bufs=2, space="PSUM")
        p3s = pool(name="p3", bufs=1, space="PSUM")
        if True:
            ident = res.tile([128, 128], F32)
            make_identity(nc, ident)
            identh = res.tile([128, 128], F16)
            make_identity(nc, identh)

            # ---- biases: preload all groups once, transposed on PE ----
            b012_all = res.tile([128, 3, 8, TWO_N], F32)
            b3_all = res.tile([16, TWO_N], F32)   # [n, l]
            for li, bsrc in enumerate((b0, b1, b2)):
                bnat = wk.tile([TWO_N, H], F32, name=f"bnat{li}", tag="bnat")
                nc.sync.dma_start(out=bnat, in_=bsrc[:, :])
                for ot in range(8):
                    pt = tps.tile([128, 128], F32, name="tpb", tag="tp")
                    nc.tensor.transpose(
                        pt[:, 0:TWO_N], bnat[:, 128 * ot:128 * (ot + 1)],
                        ident[0:TWO_N, 0:TWO_N])
                    nc.scalar.copy(b012_all[:, li, ot, :], pt[:, 0:TWO_N])
            b3nat = wk.tile([TWO_N, TWO_N], F32, name="b3nat", tag="bnat")
            nc.sync.dma_start(out=b3nat, in_=b3[:, :])
            ptb = tps.tile([128, 128], F32, name="tpb3", tag="tp")
            nc.tensor.transpose(ptb[0:TWO_N, 0:TWO_N], b3nat[:, :],
                                ident[0:TWO_N, 0:TWO_N])
            nc.scalar.copy(b3_all[:, :], ptb[0:TWO_N, 0:TWO_N])

            # resident fp16 x (gram mults + finals)
            xhall = res.tile([128, N_SUB, TWO_N * D], F16)
            # resident transposed scalar grid [256, B_LOC] fp16 (2 tiles)
            scalTa = res.tile([128, B_LOC], F16, name="scalTa")
            scalTb = res.tile([128, B_LOC], F16, name="scalTb")
            sbm_tiles = [None] * N_SUB

            def ap3(t, off, s0, n0, s1, n1):
                return bass.AP(tensor=t.tensor, offset=t.offset + off,
                               ap=[t.ap[0], [s0, n0], [s1, n1]])

            # ---------------- Gram DVE/GP part for one subtile ----------
            def gram_dve(s):
                xg = xgp.tile([128, TWO_N * D], F32, name="xg", tag="xg")
                nc.sync.dma_start(out=xg, in_=x[128 * s:128 * (s + 1), :])
                xh = xhall[:, s, :]
                nc.scalar.copy(xh, xg)
                sbm = sbmp.tile([128, 256], F16, name=f"sbm{s}", tag="sbm")
                sbm_tiles[s] = sbm
                nc.gpsimd.memset(sbm[:, :], 0.0)
                pb = pbp.tile([128, K_TRI * D], F16, name="pb", tag="pb")
                for dl in range(TWO_N):
                    npair = TWO_N - dl
                    meng = nc.vector if dl < SPLIT_DL else nc.gpsimd
                    meng.tensor_mul(
                        pb[:, CUM[dl] * D:(CUM[dl] + npair) * D],
                        xh[0:128, 0:npair * D],
                        xh[0:128, dl * D:(dl + npair) * D],
                    )
                for w in (32, 16, 8):
                    nc.vector.tensor_add(ap3(pb, 0, D, K_TRI, 1, w),
                                         ap3(pb, 0, D, K_TRI, 1, w),
                                         ap3(pb, w, D, K_TRI, 1, w))
                with nc.allow_low_precision("fp16 gram accum, ~2e-3 ok"):
                    for dl in range(TWO_N):
                        npair = TWO_N - dl
                        dst = bass.AP(tensor=sbm.tensor,
                                      offset=sbm.offset + dl,
                                      ap=[sbm.ap[0], [17, npair]])
                        nc.vector.tensor_reduce(
                            dst, ap3(pb, CUM[dl] * D, D, npair, 1, 8),
                            axis=mybir.AxisListType.X,
                            op=mybir.AluOpType.add)

            # ---- Gram PE part: transpose sbm grid -> scalTa/scalTb ----
            def gram_pe(s):
                sbm = sbm_tiles[s]
                pt = tps.tile([128, 128], F16, name="tpg", tag="tp")
                nc.tensor.transpose(pt[:, :], sbm[:, 0:128], identh)
                nc.scalar.copy(scalTa[:, 128 * s:128 * (s + 1)], pt[:, :])
                ptb2 = tps.tile([128, 128], F16, name="tpg8", tag="tp")
                nc.tensor.transpose(ptb2[:, :], sbm[:, 128:256], identh)
                nc.scalar.copy(scalTb[:, 128 * s:128 * (s + 1)], ptb2[:, :])

            # ---- final contraction for one (l, subtile) ----
            def final_unit(l, s, cexp):
                meng = nc.gpsimd if s % 4 == 3 else nc.vector
                fpool = pgp if s % 4 == 3 else pvp
                prod = fpool.tile([128, TWO_N * D], F16, name="prod2",
                                  tag="prod")
                meng.tensor_mul(prod[:, :], xhall[:, s, :], cexp[:, :])
                meng.tensor_add(prod[:, 0:512], prod[:, 0:512],
                                prod[:, 512:1024])
                meng.tensor_add(prod[:, 0:256], prod[:, 0:256],
                                prod[:, 256:512])
                meng.tensor_add(prod[:, 0:128], prod[:, 0:128],
                                prod[:, 128:256])
                fcol = finp.tile([128, D], F32, name="fcol", tag="fcol")
                meng.tensor_add(fcol[:, :], prod[:, 0:D], prod[:, D:2 * D])
                nc.sync.dma_start(
                    out=y[128 * s:128 * (s + 1), D * l:D * (l + 1)],
                    in_=fcol[:, :])

            # ---------------- gram emission (first half) ----------------
            for s in range(8):
                gram_dve(s)

            # PE warmup while the first gram subtiles flow through DVE/GP
            for i in range(230):
                wtp = tps.tile([128, 128], F16, name="warm", tag="tp")
                nc.tensor.transpose(wtp[:, :], identh, identh)

            pend_pe = []     # transposes + coeff staging (after next L0)
            pend_late = []   # cexp expansion + finals (mid-chunk)
            pend_fin = []    # finals deferred past the gram window

            def flush(lst):
                for fn in lst:
                    fn()
                lst.clear()

            def mlp_chunk(l, bt, w0t, w1h, w2h, w3t,
                          hook_l0=None, hook_l1=None):
                bs = 512 * bt
                # L0
                h0 = hp.tile([128, 8, 512], F16, name="h0", tag="h")
                for ot in range(8):
                    pt = ps.tile([128, 512], F32, name="mlp", tag="mlp")
                    nc.tensor.matmul(
                        pt[:, :], w0t[:, 0, 128 * ot:128 * (ot + 1)],
                        scalTa[:, bs:bs + 512], start=True, stop=False)
                    nc.tensor.matmul(
                        pt[:, :], w0t[:, 1, 128 * ot:128 * (ot + 1)],
                        scalTb[:, bs:bs + 512], start=False, stop=True)
                    nc.scalar.activation(
                        h0[:, ot, :], pt[:, :], TANH,
                        bias=b012_all[:, 0, ot, l:l + 1])
                if hook_l0 is not None:
                    hook_l0()
                hin = h0
                for li_idx, (li, whalves) in enumerate(((1, w1h), (2, w2h))):
                    if li_idx == 1 and hook_l1 is not None:
                        hook_l1()
                    hout = hp.tile([128, 8, 512], F16, name=f"h{li}",
                                   tag="h")
                    for ot in range(8):
                        pt = ps.tile([128, 512], F32, name="mlp",
                                     tag="mlp")
                        for kt in range(8):
                            nc.tensor.matmul(
                                pt[:, :],
                                whalves[kt // 4][:, kt % 4,
                                                 128 * ot:128 * (ot + 1)],
                                hin[:, kt, :],
                                start=(kt == 0), stop=(kt == 7))
                        nc.scalar.activation(
                            hout[:, ot, :], pt[:, :], TANH,
                            bias=b012_all[:, li, ot, l:l + 1])
                    hin = hout
                p3 = p3s.tile([16, 512], F32, name="p3", tag="p3")
                for kt in range(8):
                    nc.tensor.matmul(p3[:, :], w3t[:, kt, :],
                                     hin[:, kt, :],
                                     start=(kt == 0), stop=(kt == 7))
                csb = csbp.tile([16, 512], F32, name="csb", tag="csb")
                nc.scalar.add(csb[:, :], p3[:, :], b3_all[:, l:l + 1])

                def pe_tail(l=l, bt=bt, csb=csb):
                    cfs = []
                    for j in range(4):
                        tp = tps.tile([128, 16], F32, name="tp2", tag="tp")
                        nc.tensor.transpose(
                            tp[:, 0:16], csb[:, 128 * j:128 * (j + 1)],
                            ident[0:16, 0:16])
                        cf = cfp.tile([128, 16], F16, name="cf", tag="cf")
                        nc.scalar.copy(cf[:, :], tp[:, 0:16])
                        cfs.append(cf)

                    def late_tail(l=l, bt=bt, cfs=cfs):
                        for j in range(4):
                            cexp = cxp.tile([128, TWO_N * D], F16,
                                            name="cexp", tag="cx")
                            bcast = bass.AP(
                                tensor=cfs[j].tensor,
                                offset=cfs[j].offset,
                                ap=[cfs[j].ap[0], [1, TWO_N], [0, D]])
                            nc.scalar.copy(cexp[:, :], bcast)
                            final_unit(l, 4 * bt + j, cexp)
                    pend_late.append(late_tail)
                pend_pe.append(pe_tail)

            # ---------------- main: bt-pair-major ----------------
            for half in range(2):
                bts = (2 * half, 2 * half + 1)
                for l in range(TWO_N):
                    w0t = w0p.tile([128, 2, H], F16, name="w0t", tag="w0")
                    nc.sync.dma_start(
                        out=w0t,
                        in_=W0g[l, :, :].rearrange("(t p) m -> p t m",
                                                   p=128))
                    w1h = []
                    w2h = []
                    for hi in range(4):
                        wt = w12p.tile([128, 4, H], F16, name=f"w{hi}",
                                       tag="w12")
                        nc.sync.dma_start(
                            out=wt,
                            in_=W12[l, 512 * hi:512 * (hi + 1), :]
                            .rearrange("(t p) m -> p t m", p=128))
                        (w1h if hi < 2 else w2h).append(wt)
                    w3t = w3p.tile([128, 8, TWO_N], F16, name="w3t",
                                   tag="w3")
                    nc.sync.dma_start(
                        out=w3t,
                        in_=W3[l, :, :].rearrange("(t p) m -> p t m",
                                                  p=128))

                    defer = (half == 0 and l < 3)
                    if l == 0:
                        for s in range(8 * half, 8 * half + 4):
                            gram_pe(s)
                    mlp_chunk(l, bts[0], w0t, w1h, w2h, w3t,
                              hook_l0=lambda: flush(pend_pe),
                              hook_l1=(None if defer else
                                       lambda: flush(pend_late)))
                    if l == 0:
                        for s in range(8 * half + 4, 8 * half + 8):
                            gram_pe(s)
                    if defer:
                        pend_fin.extend(pend_late)
                        pend_late.clear()
                    elif half == 0 and 3 <= l <= 8 and pend_fin:
                        pend_fin.pop(0)()
                    mlp_chunk(l, bts[1], w0t, w1h, w2h, w3t,
                              hook_l0=lambda: flush(pend_pe),
                              hook_l1=(None if defer else
                                       lambda: flush(pend_late)))
                    if half == 0 and l == 0:
                        for s in range(8, N_SUB):
                            gram_dve(s)
                    if half == 1 and l == TWO_N - 1:
                        flush(pend_pe)
                        flush(pend_late)

            flush(pend_pe)
            flush(pend_late)

    nc.finalize()
    return nc


_NC = None


def build_in_maps(x, W0, b0, W1, b1, W2, b2, W3, b3):
    x = np.ascontiguousarray(np.asarray(x, dtype=np.float32))
    # Fold W0 over the symmetric scalar pairs: the kernel materializes the
    # upper triangle of the (n,m) grid only; mirror rows zeroed.
    W0f = np.asarray(W0, np.float32).reshape(TWO_N, TWO_N, TWO_N, H).copy()
    for n in range(TWO_N):
        for m in range(n + 1, TWO_N):
            W0f[:, n, m, :] += W0f[:, m, n, :]
            W0f[:, m, n, :] = 0.0
    W0t = W0f.reshape(TWO_N, 256, H)
    f16 = np.float16
    W12c = np.concatenate([np.asarray(W1, np.float32),
                           np.asarray(W2, np.float32)], axis=1)
    shared = {
        "W0g": np.ascontiguousarray(W0t.astype(f16)),
        "W12": np.ascontiguousarray(W12c.astype(f16)),
        "W3": np.ascontiguousarray(np.asarray(W3, np.float32).astype(f16)),
        "b0": np.ascontiguousarray(np.asarray(b0, np.float32)),
        "b1": np.ascontiguousarray(np.asarray(b1, np.float32)),
        "b2": np.ascontiguousarray(np.asarray(b2, np.float32)),
        "b3": np.ascontiguousarray(np.asarray(b3, np.float32)),
    }
    in_maps = []
    for c in range(N_CORES):
        m = dict(shared)
        m["x"] = x[B_LOC * c:B_LOC * (c + 1), :]
        in_maps.append(m)
    return in_maps


def kernel(x, W0, b0, W1, b1, W2, b2, W3, b3):
    global _NC
    if _NC is None:
        _NC = _build_program()
    in_maps = build_in_maps(x, W0, b0, W1, b1, W2, b2, W3, b3)
    res = run_bass_kernel_spmd(_NC, in_maps, list(range(N_CORES)))
    return np.concatenate([res.results[c]["y"] for c in range(N_CORES)],
                          axis=0)
